# revision 35
# baseline (speedup 1.0000x reference)
"""Distributed Trainium2 Bass kernel for nn_Attention_14955076125142.

Math (reference):
    k_enc = relu(query @ W0.T + b0)
    q_enc = relu(key  @ W1.T + b1)
    energies = rowsum(k_enc * (q_enc @ Wa.T + ba))      # (N,)
    alpha = softmax(energies)                           # (1, N)
    out = alpha @ value                                 # (1, F)

Strategy:
    Shard N=65536 rows across 8 NeuronCores (8192 rows each); replicate
    weights.  Each core computes its shard's softmax partials (running
    per-partition max m_p, sum-exp s_p, and exp-weighted value rows c_p)
    with a flash-attention-style online update fused into the main loop;
    a final on-device reduction collapses partitions, and the 8 tiny
    per-core partials are combined exactly on the host.

    Layouts: L2 (q_enc) runs "transposed" ([feature, row]) off the
    host-pre-transposed key; L1/L3 run "natural" ([row, feature]) with
    host-pre-transposed query / q_encT as the stationary operand, so the
    energies rowsum is a single DVE tensor_tensor_reduce reading the L3
    PSUM directly.  All matmuls are float32r (full-rate PE on fp32 data).
"""

import numpy as np

N_GLOBAL = 65536
F = 1024
N_CORES = 8
N_LOC = N_GLOBAL // N_CORES  # 8192
P = 128
RB = 512                     # rows per block
KC = F // P                  # contraction chunks (8)
JC = F // P                  # out-feature chunks (8)
NEG_BIG = -1.0e30


def _build(nloc=N_LOC, rb=RB, has_bias=False):
    import concourse.bacc as bacc
    import concourse.tile as tile
    import concourse.mybir as mybir
    from concourse.tile_rust import add_dep_helper

    def _raw(bi):
        return bi.ins if hasattr(bi, "ins") else bi

    dt = mybir.dt
    f32 = dt.float32
    mdt = dt.float32r   # matmul-operand dtype
    AF = mybir.ActivationFunctionType
    AX = mybir.AxisListType
    OP = mybir.AluOpType
    nb = nloc // rb
    tpb = rb // P  # row tiles per block (4)

    nc = bacc.Bacc("TRN2", target_bir_lowering=False, debug=False,
                   num_devices=N_CORES)

    qt = nc.dram_tensor("qt", [F, nloc], mdt, kind="ExternalInput")
    kt = nc.dram_tensor("kt", [F, nloc], mdt, kind="ExternalInput")
    v = nc.dram_tensor("v", [nloc, F], f32, kind="ExternalInput")
    w0t = nc.dram_tensor("w0t", [F, F], mdt, kind="ExternalInput")
    w1t = nc.dram_tensor("w1t", [F, F], mdt, kind="ExternalInput")
    wat = nc.dram_tensor("wat", [F, F], mdt, kind="ExternalInput")
    b0 = nc.dram_tensor("b0", [F], mdt, kind="ExternalInput")
    b1 = nc.dram_tensor("b1", [F], f32, kind="ExternalInput")
    ba = nc.dram_tensor("ba", [F], mdt, kind="ExternalInput")
    ones_d = nc.dram_tensor("ones1", [1, P], mdt, kind="ExternalInput")
    outc = nc.dram_tensor("outc", [P, F], f32, kind="ExternalOutput")
    outs = nc.dram_tensor("outs", [P, 2], f32, kind="ExternalOutput")

    with tile.TileContext(nc) as tc:
        with (
            tc.tile_pool(name="wpool", bufs=1) as wpool,
            tc.tile_pool(name="cpool", bufs=1) as cpool,
            tc.tile_pool(name="ktp", bufs=2) as ktp,
            tc.tile_pool(name="qep", bufs=2) as qep,
            tc.tile_pool(name="qt4p", bufs=2) as qt4p,
            tc.tile_pool(name="kencp", bufs=2) as kencp,
            tc.tile_pool(name="vtp", bufs=2 if has_bias else 3) as vtp,
            tc.tile_pool(name="smol", bufs=2) as smol,
            tc.tile_pool(name="scrp", bufs=1) as scrp,
            tc.tile_pool(name="ps", bufs=4, space="PSUM") as psp,
            tc.tile_pool(name="psL2", bufs=3, space="PSUM") as psL2,
        ):
            # ---- weights / constants ----
            # all weights in 2MB-half tiles: large DMAs keep HBM at full
            # rate, and half-tile deps let each matmul group start as soon
            # as its half has landed.  Order: kt0, w1 halves (L2 of block 0),
            # kt1 (L2 of block 1 fills the w0/wa wait), w0 halves, wa halves.
            w1_t = [wpool.tile([P, KC, 512], mdt, tag=f"w1_{h}",
                               name=f"w1_{h}") for h in range(2)]
            w0_t = [wpool.tile([P, KC, 512], mdt, tag=f"w0_{h}",
                               name=f"w0_{h}") for h in range(2)]
            wa_t = [wpool.tile([P, KC, 512], mdt, tag=f"wa_{h}",
                               name=f"wa_{h}") for h in range(2)]
            kt_b0 = ktp.tile([P, KC, rb], mdt, tag="kt", name="kt_b0")
            kt_b1 = ktp.tile([P, KC, rb], mdt, tag="kt", name="kt_b1")
            chain = []
            chain.append(nc.sync.dma_start(
                kt_b0[:], kt.ap()[:, 0:rb].rearrange("(c p) i -> p c i", p=P)))
            for h in range(2):
                chain.append(nc.sync.dma_start(
                    w1_t[h][:],
                    w1t.ap()[:, h * 512:(h + 1) * 512]
                        .rearrange("(c p) j -> p c j", p=P)))
            chain.append(nc.sync.dma_start(
                kt_b1[:], kt.ap()[:, rb:2 * rb].rearrange("(c p) i -> p c i", p=P)))
            for h in range(2):
                chain.append(nc.sync.dma_start(
                    w0_t[h][:],
                    w0t.ap()[:, h * 512:(h + 1) * 512]
                        .rearrange("(c p) j -> p c j", p=P)))
            for h in range(2):
                chain.append(nc.sync.dma_start(
                    wa_t[h][:],
                    wat.ap()[:, h * 512:(h + 1) * 512]
                        .rearrange("(c p) j -> p c j", p=P)))
            for a, b2 in zip(chain, chain[1:]):
                add_dep_helper(_raw(b2), _raw(a), False, "startup DMA order")
            wa_dmas = [chain[-1]]
            late_dmas = []   # block-0 input DMAs to gate behind the weights

            b1_sb = cpool.tile([P, JC], f32, tag="b1")
            nc.gpsimd.dma_start(b1_sb[:], b1.ap().rearrange("(c p) -> p c", p=P))
            if has_bias:
                onesr = cpool.tile([1, P], mdt, tag="onesr")
                nc.gpsimd.dma_start(onesr[:], ones_d.ap())
                b0_sb = cpool.tile([1, F], mdt, tag="b0r")
                ba_sb = cpool.tile([1, F], mdt, tag="bar")
                nc.gpsimd.dma_start(b0_sb[:], b0.ap())
                nc.gpsimd.dma_start(ba_sb[:], ba.ap())

            # online-softmax state (ping-pong pairs; no in-place DVE ops)
            m_ab = [cpool.tile([P, 1], f32, tag="m_a", name="m_a"),
                    cpool.tile([P, 1], f32, tag="m_b", name="m_b")]
            s_ab = [cpool.tile([P, 1], f32, tag="s_a", name="s_a"),
                    cpool.tile([P, 1], f32, tag="s_b", name="s_b")]
            c_ab = [cpool.tile([P, F], f32, tag="c_a", name="c_a"),
                    cpool.tile([P, F], f32, tag="c_b", name="c_b")]
            nc.gpsimd.memset(m_ab[0][:], NEG_BIG)
            nc.gpsimd.memset(s_ab[0][:], 0.0)
            nc.gpsimd.memset(c_ab[0][:], 0.0)

            qencs = {}

            def emit_t4_block(b):
                bs = b * rb
                qenc = qencs.pop(b)
                for t4 in range(tpb):
                    t_glob = b * tpb + t4
                    off = bs + t4 * P
                    qt_4 = qt4p.tile([P, KC, P], mdt, tag="qt4")
                    d1 = nc.sync.dma_start(
                        qt_4[:],
                        qt.ap()[:, off:off + P].rearrange("(c p) i -> p c i", p=P))
                    vt = vtp.tile([P, F], f32, tag="vt")
                    d2 = nc.sync.dma_start(vt[:], v.ap()[off:off + P, :])
                    if b == 0:
                        if t4 > 0:
                            late_dmas.append(d1)
                        late_dmas.append(d2)

                    # ---- L1 natural: kenc = relu(q @ W0.T [+ b0])
                    kenc = kencp.tile([P, F], f32, tag="kenc")
                    for jh in range(2):
                        ps1 = psp.tile([P, 512], f32, tag="ps")
                        for kc in range(KC):
                            nc.tensor.matmul(
                                ps1[:],
                                qt_4[:, kc, :],
                                w0_t[jh][:, kc, :],
                                start=(kc == 0),
                                stop=(kc == KC - 1 and not has_bias),
                            )
                        if has_bias:
                            nc.tensor.matmul(ps1[:], onesr[:],
                                             b0_sb[:, jh * 512:(jh + 1) * 512],
                                             start=False, stop=True)
                        nc.scalar.activation(
                            kenc[:, jh * 512:(jh + 1) * 512], ps1[:], AF.Relu)

                    # ---- L3 natural: attn psum = q_enc @ Wa.T; fused energies
                    e_tmp = smol.tile([P, 1], f32, tag="e_tmp")
                    e_tmp2 = smol.tile([P, 1], f32, tag="e_tmp2")
                    ecol = smol.tile([P, 1], f32, tag="ecol")
                    for jh in range(2):
                        ps3 = psp.tile([P, 512], f32, tag="ps")
                        for kc in range(KC):
                            nc.tensor.matmul(
                                ps3[:],
                                qenc[:, kc, t4 * P:(t4 + 1) * P],
                                wa_t[jh][:, kc, :],
                                start=(kc == 0),
                                stop=(kc == KC - 1 and not has_bias),
                            )
                        if has_bias:
                            nc.tensor.matmul(ps3[:], onesr[:],
                                             ba_sb[:, jh * 512:(jh + 1) * 512],
                                             start=False, stop=True)
                        # energies partial: rowsum(kenc * attn) over this half
                        pscr = scrp.tile([P, 512], f32, tag="pscr")
                        nc.vector.scalar_tensor_tensor(
                            out=pscr[:],
                            in0=kenc[:, jh * 512:(jh + 1) * 512],
                            scalar=1.0,
                            in1=ps3[:],
                            op0=OP.mult, op1=OP.mult,
                            accum_out=(e_tmp[:] if jh == 0 else e_tmp2[:]),
                        )
                    nc.vector.tensor_add(ecol[:], e_tmp[:], e_tmp2[:])

                    # ---- per-tile online softmax update ----
                    m_old = m_ab[t_glob % 2]
                    m_new = m_ab[(t_glob + 1) % 2]
                    s_old = s_ab[t_glob % 2]
                    s_new = s_ab[(t_glob + 1) % 2]
                    c_old = c_ab[t_glob % 2]
                    c_new = c_ab[(t_glob + 1) % 2]
                    nc.vector.tensor_max(m_new[:], m_old[:], ecol[:])
                    dm = smol.tile([P, 1], f32, tag="dm")
                    nc.vector.tensor_sub(dm[:], m_old[:], m_new[:])
                    sc = smol.tile([P, 1], f32, tag="sc")
                    nc.scalar.activation(sc[:], dm[:], AF.Exp)
                    negm = smol.tile([P, 1], f32, tag="negm")
                    nc.vector.tensor_scalar_mul(negm[:], m_new[:], -1.0)
                    wv = smol.tile([P, 1], f32, tag="wv")
                    nc.scalar.activation(wv[:], ecol[:], AF.Exp, bias=negm[:])
                    nc.vector.scalar_tensor_tensor(
                        out=s_new[:], in0=s_old[:], scalar=sc[:], in1=wv[:],
                        op0=OP.mult, op1=OP.add)
                    ctmp = smol.tile([P, F], f32, tag="ctmp", bufs=1,
                                     name=f"ctmp_{b}_{t4}")
                    nc.vector.tensor_scalar_mul(ctmp[:], c_old[:], sc[:])
                    nc.vector.scalar_tensor_tensor(
                        out=c_new[:], in0=vt[:], scalar=wv[:], in1=ctmp[:],
                        op0=OP.mult, op1=OP.add)

            for b in range(nb):
                bs = b * rb
                if b == 0:
                    kt_t = kt_b0
                elif b == 1:
                    kt_t = kt_b1
                else:
                    kt_t = ktp.tile([P, KC, rb], mdt, tag="kt",
                                    name=f"kt_{b}")
                    nc.sync.dma_start(
                        kt_t[:],
                        kt.ap()[:, bs:bs + rb].rearrange("(c p) i -> p c i", p=P))
                qenc = qep.tile([P, KC, rb], mdt, tag="qe")
                qencs[b] = qenc

                # ---- L2 transposed: qencT = relu(W1T.T @ ktT + b1) ----
                for jc in range(JC):
                    ps = psL2.tile([P, rb], f32, tag="ps2")
                    for kc in range(KC):
                        nc.tensor.matmul(
                            ps[:],
                            w1_t[jc // 4][:, kc, (jc % 4) * P:(jc % 4 + 1) * P],
                            kt_t[:, kc, :],
                            start=(kc == 0), stop=(kc == KC - 1),
                        )
                    nc.scalar.activation(qenc[:, jc, :], ps[:], AF.Relu,
                                         bias=b1_sb[:, jc:jc + 1])

                # one-block lookahead: run the previous block's row tiles
                # while this block's L2 (and block-0/1 weight DMAs) proceed
                if b >= 1:
                    emit_t4_block(b - 1)
            emit_t4_block(nb - 1)

            # gate the bulky block-0 input loads behind the weight loads
            for d in late_dmas:
                add_dep_helper(_raw(d), _raw(wa_dmas[-1]), False,
                               "gate block-0 inputs behind weights")

            # ---- ship per-partition partials; host does the reduce ----
            fin = (nb * tpb) % 2
            nc.sync.dma_start(outc.ap(), c_ab[fin][:])
            st2 = cpool.tile([P, 2], f32, tag="st2")
            nc.vector.tensor_copy(st2[:, 0:1], m_ab[fin][:])
            nc.vector.tensor_copy(st2[:, 1:2], s_ab[fin][:])
            nc.sync.dma_start(outs.ap(), st2[:])

    nc.compile()
    return nc


def _prepare(inputs, nloc=N_LOC):
    """Host-side sharding/layout prep. Returns (nc, in_maps)."""
    key = np.ascontiguousarray(np.asarray(inputs["key"], dtype=np.float32))
    query = np.ascontiguousarray(np.asarray(inputs["query"], dtype=np.float32))
    value = np.ascontiguousarray(np.asarray(inputs["value"], dtype=np.float32))
    w0t = np.ascontiguousarray(np.asarray(inputs["W0"], dtype=np.float32).T)
    w1t = np.ascontiguousarray(np.asarray(inputs["W1"], dtype=np.float32).T)
    wat = np.ascontiguousarray(np.asarray(inputs["Wa"], dtype=np.float32).T)
    b0 = np.ascontiguousarray(np.asarray(inputs["b0"], dtype=np.float32))
    b1 = np.ascontiguousarray(np.asarray(inputs["b1"], dtype=np.float32))
    ba = np.ascontiguousarray(np.asarray(inputs["ba"], dtype=np.float32))

    has_bias = bool(np.any(b0 != 0) or np.any(ba != 0))

    qT = np.ascontiguousarray(query.T)  # (F, N)
    kT = np.ascontiguousarray(key.T)

    in_maps = []
    for c in range(N_CORES):
        sl = slice(c * nloc, (c + 1) * nloc)
        in_maps.append({
            "qt": np.ascontiguousarray(qT[:, sl]),
            "kt": np.ascontiguousarray(kT[:, sl]),
            "v": np.ascontiguousarray(value[sl]),
            "w0t": w0t, "w1t": w1t, "wat": wat,
            "b0": b0, "b1": b1, "ba": ba,
            "ones1": np.ones((1, P), dtype=np.float32),
        })
    nc = _build(nloc=nloc, has_bias=has_bias)
    return nc, in_maps


def _combine(outs):
    """Combine per-core per-partition partials into the (1, F) context.

    Each core returns c_p (P, F), and (m_p, s_p) in an (P, 2) stats block;
    the exact log-sum-exp merge over all 8*P partials runs here in float64.
    """
    c = np.concatenate([o[0].astype(np.float64) for o in outs], axis=0)
    st = np.concatenate([o[1].astype(np.float64) for o in outs], axis=0)
    m, s = st[:, 0], st[:, 1]
    M = m.max()
    scale = np.exp(m - M)
    S = float((s * scale).sum())
    C = (c * scale[:, None]).sum(axis=0)
    return (C / S)[None, :].astype(np.float32)


def kernel(**inputs):
    from concourse import bass_utils
    nc, in_maps = _prepare(inputs)
    res = bass_utils.run_bass_kernel_spmd(
        nc, in_maps, core_ids=list(range(N_CORES)))
    return _combine([(r["outc"], r["outs"]) for r in res.results])


# revision 36
# speedup vs baseline: 1.1889x; 1.1889x over previous
"""Distributed Trainium2 Bass kernel for nn_Attention_14955076125142.

Math (reference):
    k_enc = relu(query @ W0.T + b0)
    q_enc = relu(key  @ W1.T + b1)
    energies = rowsum(k_enc * (q_enc @ Wa.T + ba))      # (N,)
    alpha = softmax(energies)                           # (1, N)
    out = alpha @ value                                 # (1, F)

Strategy:
    Shard N=65536 rows across 8 NeuronCores (8192 rows each); replicate
    weights.  Each core computes its shard's softmax partials (running
    per-partition max m_p, sum-exp s_p, and exp-weighted value rows c_p)
    with a flash-attention-style online update fused into the main loop;
    a final on-device reduction collapses partitions, and the 8 tiny
    per-core partials are combined exactly on the host.

    Layouts: L2 (q_enc) runs "transposed" ([feature, row]) off the
    host-pre-transposed key; L1/L3 run "natural" ([row, feature]) with
    host-pre-transposed query / q_encT as the stationary operand, so the
    energies rowsum is a single DVE tensor_tensor_reduce reading the L3
    PSUM directly.  All matmuls are float32r (full-rate PE on fp32 data).
"""

import numpy as np

N_GLOBAL = 65536
F = 1024
N_CORES = 8
N_LOC = N_GLOBAL // N_CORES  # 8192
P = 128
RB = 512                     # rows per block
KC = F // P                  # contraction chunks (8)
JC = F // P                  # out-feature chunks (8)
NEG_BIG = -1.0e30


def _build(nloc=N_LOC, rb=RB, has_bias=False):
    import concourse.bacc as bacc
    import concourse.tile as tile
    import concourse.mybir as mybir
    from concourse.tile_rust import add_dep_helper

    def _raw(bi):
        return bi.ins if hasattr(bi, "ins") else bi

    dt = mybir.dt
    f32 = dt.float32
    mdt = dt.float32r   # matmul-operand dtype
    AF = mybir.ActivationFunctionType
    AX = mybir.AxisListType
    OP = mybir.AluOpType
    nb = nloc // rb
    tpb = rb // P  # row tiles per block (4)

    nc = bacc.Bacc("TRN2", target_bir_lowering=False, debug=False,
                   num_devices=N_CORES)

    qt = nc.dram_tensor("qt", [F, nloc], mdt, kind="ExternalInput")
    kt = nc.dram_tensor("kt", [F, nloc], mdt, kind="ExternalInput")
    v = nc.dram_tensor("v", [nloc, F], f32, kind="ExternalInput")
    w0t = nc.dram_tensor("w0t", [F, F], mdt, kind="ExternalInput")
    w1t = nc.dram_tensor("w1t", [F, F], mdt, kind="ExternalInput")
    wat = nc.dram_tensor("wat", [F, F], mdt, kind="ExternalInput")
    b0 = nc.dram_tensor("b0", [F], mdt, kind="ExternalInput")
    b1 = nc.dram_tensor("b1", [F], f32, kind="ExternalInput")
    ba = nc.dram_tensor("ba", [F], mdt, kind="ExternalInput")
    ones_d = nc.dram_tensor("ones1", [1, P], mdt, kind="ExternalInput")
    outc = nc.dram_tensor("outc", [P, F], f32, kind="ExternalOutput")
    outs = nc.dram_tensor("outs", [P, 2], f32, kind="ExternalOutput")

    with tile.TileContext(nc) as tc:
        with (
            tc.tile_pool(name="wpool", bufs=1) as wpool,
            tc.tile_pool(name="cpool", bufs=1) as cpool,
            tc.tile_pool(name="ktp", bufs=2) as ktp,
            tc.tile_pool(name="qep", bufs=2) as qep,
            tc.tile_pool(name="qt4p", bufs=2) as qt4p,
            tc.tile_pool(name="kencp", bufs=2) as kencp,
            tc.tile_pool(name="vtp", bufs=2 if has_bias else 3) as vtp,
            tc.tile_pool(name="smol", bufs=2) as smol,
            tc.tile_pool(name="scrp", bufs=1) as scrp,
            tc.tile_pool(name="ps", bufs=5, space="PSUM") as psp,
            tc.tile_pool(name="psL2", bufs=3, space="PSUM") as psL2,
        ):
            # ---- weights / constants ----
            # all weights in 2MB-half tiles: large DMAs keep HBM at full
            # rate, and half-tile deps let each matmul group start as soon
            # as its half has landed.  Order: kt0, w1 halves (L2 of block 0),
            # kt1 (L2 of block 1 fills the w0/wa wait), w0 halves, wa halves.
            w1_t = [wpool.tile([P, KC, 512], mdt, tag=f"w1_{h}",
                               name=f"w1_{h}") for h in range(2)]
            w0_t = [wpool.tile([P, KC, 512], mdt, tag=f"w0_{h}",
                               name=f"w0_{h}") for h in range(2)]
            wa_t = [wpool.tile([P, KC, 512], mdt, tag=f"wa_{h}",
                               name=f"wa_{h}") for h in range(2)]
            kt_b0 = ktp.tile([P, KC, rb], mdt, tag="kt", name="kt_b0")
            kt_b1 = ktp.tile([P, KC, rb], mdt, tag="kt", name="kt_b1")
            chain = []
            chain.append(nc.sync.dma_start(
                kt_b0[:], kt.ap()[:, 0:rb].rearrange("(c p) i -> p c i", p=P)))
            for h in range(2):
                chain.append(nc.sync.dma_start(
                    w1_t[h][:],
                    w1t.ap()[:, h * 512:(h + 1) * 512]
                        .rearrange("(c p) j -> p c j", p=P)))
            chain.append(nc.sync.dma_start(
                kt_b1[:], kt.ap()[:, rb:2 * rb].rearrange("(c p) i -> p c i", p=P)))
            for h in range(2):
                chain.append(nc.sync.dma_start(
                    w0_t[h][:],
                    w0t.ap()[:, h * 512:(h + 1) * 512]
                        .rearrange("(c p) j -> p c j", p=P)))
            for h in range(2):
                chain.append(nc.sync.dma_start(
                    wa_t[h][:],
                    wat.ap()[:, h * 512:(h + 1) * 512]
                        .rearrange("(c p) j -> p c j", p=P)))
            for a, b2 in zip(chain, chain[1:]):
                add_dep_helper(_raw(b2), _raw(a), False, "startup DMA order")
            wa_dmas = [chain[-1]]
            late_dmas = []   # block-0 input DMAs to gate behind the weights

            b1_sb = cpool.tile([P, JC], f32, tag="b1")
            nc.gpsimd.dma_start(b1_sb[:], b1.ap().rearrange("(c p) -> p c", p=P))
            if has_bias:
                onesr = cpool.tile([1, P], mdt, tag="onesr")
                nc.gpsimd.dma_start(onesr[:], ones_d.ap())
                b0_sb = cpool.tile([1, F], mdt, tag="b0r")
                ba_sb = cpool.tile([1, F], mdt, tag="bar")
                nc.gpsimd.dma_start(b0_sb[:], b0.ap())
                nc.gpsimd.dma_start(ba_sb[:], ba.ap())

            # online-softmax state (ping-pong pairs; no in-place DVE ops)
            m_ab = [cpool.tile([P, 1], f32, tag="m_a", name="m_a"),
                    cpool.tile([P, 1], f32, tag="m_b", name="m_b")]
            s_ab = [cpool.tile([P, 1], f32, tag="s_a", name="s_a"),
                    cpool.tile([P, 1], f32, tag="s_b", name="s_b")]
            c_ab = [cpool.tile([P, F], f32, tag="c_a", name="c_a"),
                    cpool.tile([P, F], f32, tag="c_b", name="c_b")]
            nc.gpsimd.memset(m_ab[0][:], NEG_BIG)
            nc.gpsimd.memset(s_ab[0][:], 0.0)
            nc.gpsimd.memset(c_ab[0][:], 0.0)

            qencs = {}

            def emit_t4_block(b):
                bs = b * rb
                qenc = qencs.pop(b)
                for t4 in range(tpb):
                    t_glob = b * tpb + t4
                    off = bs + t4 * P
                    qt_4 = qt4p.tile([P, KC, P], mdt, tag="qt4")
                    d1 = nc.sync.dma_start(
                        qt_4[:],
                        qt.ap()[:, off:off + P].rearrange("(c p) i -> p c i", p=P))
                    vt = vtp.tile([P, F], f32, tag="vt")
                    d2 = nc.sync.dma_start(vt[:], v.ap()[off:off + P, :])
                    if b == 0:
                        if t4 > 0:
                            late_dmas.append(d1)
                        late_dmas.append(d2)

                    # ---- L1 natural: kenc = relu(q @ W0.T [+ b0])
                    kenc = kencp.tile([P, F], f32, tag="kenc")
                    for jh in range(2):
                        ps1 = psp.tile([P, 512], f32, tag="ps")
                        for kc in range(KC):
                            nc.tensor.matmul(
                                ps1[:],
                                qt_4[:, kc, :],
                                w0_t[jh][:, kc, :],
                                start=(kc == 0),
                                stop=(kc == KC - 1 and not has_bias),
                            )
                        if has_bias:
                            nc.tensor.matmul(ps1[:], onesr[:],
                                             b0_sb[:, jh * 512:(jh + 1) * 512],
                                             start=False, stop=True)
                        nc.scalar.activation(
                            kenc[:, jh * 512:(jh + 1) * 512], ps1[:], AF.Relu)

                    # ---- L3 natural: attn psum = q_enc @ Wa.T; fused energies
                    e_tmp = smol.tile([P, 1], f32, tag="e_tmp")
                    e_tmp2 = smol.tile([P, 1], f32, tag="e_tmp2")
                    ecol = smol.tile([P, 1], f32, tag="ecol")
                    for jh in range(2):
                        ps3 = psp.tile([P, 512], f32, tag="ps")
                        for kc in range(KC):
                            nc.tensor.matmul(
                                ps3[:],
                                qenc[:, kc, t4 * P:(t4 + 1) * P],
                                wa_t[jh][:, kc, :],
                                start=(kc == 0),
                                stop=(kc == KC - 1 and not has_bias),
                            )
                        if has_bias:
                            nc.tensor.matmul(ps3[:], onesr[:],
                                             ba_sb[:, jh * 512:(jh + 1) * 512],
                                             start=False, stop=True)
                        # energies partial: rowsum(kenc * attn) over this half
                        pscr = scrp.tile([P, 512], f32, tag="pscr")
                        nc.vector.scalar_tensor_tensor(
                            out=pscr[:],
                            in0=kenc[:, jh * 512:(jh + 1) * 512],
                            scalar=1.0,
                            in1=ps3[:],
                            op0=OP.mult, op1=OP.mult,
                            accum_out=(e_tmp[:] if jh == 0 else e_tmp2[:]),
                        )
                    nc.vector.tensor_add(ecol[:], e_tmp[:], e_tmp2[:])

                    # ---- per-tile online softmax update ----
                    m_old = m_ab[t_glob % 2]
                    m_new = m_ab[(t_glob + 1) % 2]
                    s_old = s_ab[t_glob % 2]
                    s_new = s_ab[(t_glob + 1) % 2]
                    c_old = c_ab[t_glob % 2]
                    c_new = c_ab[(t_glob + 1) % 2]
                    nc.vector.tensor_max(m_new[:], m_old[:], ecol[:])
                    dm = smol.tile([P, 1], f32, tag="dm")
                    nc.vector.tensor_sub(dm[:], m_old[:], m_new[:])
                    sc = smol.tile([P, 1], f32, tag="sc")
                    nc.scalar.activation(sc[:], dm[:], AF.Exp)
                    negm = smol.tile([P, 1], f32, tag="negm")
                    nc.vector.tensor_scalar_mul(negm[:], m_new[:], -1.0)
                    wv = smol.tile([P, 1], f32, tag="wv")
                    nc.scalar.activation(wv[:], ecol[:], AF.Exp, bias=negm[:])
                    nc.vector.scalar_tensor_tensor(
                        out=s_new[:], in0=s_old[:], scalar=sc[:], in1=wv[:],
                        op0=OP.mult, op1=OP.add)
                    ctmp = smol.tile([P, F], f32, tag="ctmp", bufs=1,
                                     name=f"ctmp_{b}_{t4}")
                    nc.vector.tensor_scalar_mul(ctmp[:], c_old[:], sc[:])
                    nc.vector.scalar_tensor_tensor(
                        out=c_new[:], in0=vt[:], scalar=wv[:], in1=ctmp[:],
                        op0=OP.mult, op1=OP.add)

            for b in range(nb):
                bs = b * rb
                if b == 0:
                    kt_t = kt_b0
                elif b == 1:
                    kt_t = kt_b1
                else:
                    kt_t = ktp.tile([P, KC, rb], mdt, tag="kt",
                                    name=f"kt_{b}")
                    nc.sync.dma_start(
                        kt_t[:],
                        kt.ap()[:, bs:bs + rb].rearrange("(c p) i -> p c i", p=P))
                qenc = qep.tile([P, KC, rb], mdt, tag="qe")
                qencs[b] = qenc

                # ---- L2 transposed: qencT = relu(W1T.T @ ktT + b1) ----
                for jc in range(JC):
                    ps = psL2.tile([P, rb], f32, tag="ps2")
                    for kc in range(KC):
                        nc.tensor.matmul(
                            ps[:],
                            w1_t[jc // 4][:, kc, (jc % 4) * P:(jc % 4 + 1) * P],
                            kt_t[:, kc, :],
                            start=(kc == 0), stop=(kc == KC - 1),
                        )
                    nc.scalar.activation(qenc[:, jc, :], ps[:], AF.Relu,
                                         bias=b1_sb[:, jc:jc + 1])

                # one-block lookahead: run the previous block's row tiles
                # while this block's L2 (and block-0/1 weight DMAs) proceed
                if b >= 1:
                    emit_t4_block(b - 1)
            emit_t4_block(nb - 1)

            # gate the bulky block-0 input loads behind the weight loads
            for d in late_dmas:
                add_dep_helper(_raw(d), _raw(wa_dmas[-1]), False,
                               "gate block-0 inputs behind weights")

            # ---- ship per-partition partials; host does the reduce ----
            fin = (nb * tpb) % 2
            nc.sync.dma_start(outc.ap(), c_ab[fin][:])
            st2 = cpool.tile([P, 2], f32, tag="st2")
            nc.vector.tensor_copy(st2[:, 0:1], m_ab[fin][:])
            nc.vector.tensor_copy(st2[:, 1:2], s_ab[fin][:])
            nc.sync.dma_start(outs.ap(), st2[:])

    nc.compile()
    return nc


def _prepare(inputs, nloc=N_LOC):
    """Host-side sharding/layout prep. Returns (nc, in_maps)."""
    key = np.ascontiguousarray(np.asarray(inputs["key"], dtype=np.float32))
    query = np.ascontiguousarray(np.asarray(inputs["query"], dtype=np.float32))
    value = np.ascontiguousarray(np.asarray(inputs["value"], dtype=np.float32))
    w0t = np.ascontiguousarray(np.asarray(inputs["W0"], dtype=np.float32).T)
    w1t = np.ascontiguousarray(np.asarray(inputs["W1"], dtype=np.float32).T)
    wat = np.ascontiguousarray(np.asarray(inputs["Wa"], dtype=np.float32).T)
    b0 = np.ascontiguousarray(np.asarray(inputs["b0"], dtype=np.float32))
    b1 = np.ascontiguousarray(np.asarray(inputs["b1"], dtype=np.float32))
    ba = np.ascontiguousarray(np.asarray(inputs["ba"], dtype=np.float32))

    has_bias = bool(np.any(b0 != 0) or np.any(ba != 0))

    qT = np.ascontiguousarray(query.T)  # (F, N)
    kT = np.ascontiguousarray(key.T)

    in_maps = []
    for c in range(N_CORES):
        sl = slice(c * nloc, (c + 1) * nloc)
        in_maps.append({
            "qt": np.ascontiguousarray(qT[:, sl]),
            "kt": np.ascontiguousarray(kT[:, sl]),
            "v": np.ascontiguousarray(value[sl]),
            "w0t": w0t, "w1t": w1t, "wat": wat,
            "b0": b0, "b1": b1, "ba": ba,
            "ones1": np.ones((1, P), dtype=np.float32),
        })
    nc = _build(nloc=nloc, has_bias=has_bias)
    return nc, in_maps


def _combine(outs):
    """Combine per-core per-partition partials into the (1, F) context.

    Each core returns c_p (P, F), and (m_p, s_p) in an (P, 2) stats block;
    the exact log-sum-exp merge over all 8*P partials runs here in float64.
    """
    c = np.concatenate([o[0].astype(np.float64) for o in outs], axis=0)
    st = np.concatenate([o[1].astype(np.float64) for o in outs], axis=0)
    m, s = st[:, 0], st[:, 1]
    M = m.max()
    scale = np.exp(m - M)
    S = float((s * scale).sum())
    C = (c * scale[:, None]).sum(axis=0)
    return (C / S)[None, :].astype(np.float32)


def kernel(**inputs):
    from concourse import bass_utils
    nc, in_maps = _prepare(inputs)
    res = bass_utils.run_bass_kernel_spmd(
        nc, in_maps, core_ids=list(range(N_CORES)))
    return _combine([(r["outc"], r["outs"]) for r in res.results])
